# revision 42
# baseline (speedup 1.0000x reference)
"""Chamfer loss kernel for Trainium2 (8 NeuronCores, Bass/Tile).

Problem: x, y of shape [8192, 128] fp32.
  dist[i,j] = max(||x_i||^2 + ||y_j||^2 - 2 x_i.y_j, 0)
  loss = (sum_j min_i dist[i,j] + sum_i min_j dist[i,j]) / 8192

Sharding: x rows are split across the 8 cores (1024 rows each); every core
holds all of y. Each core computes its [1024, 8192] distance tile via PE
matmuls (K = 128 features on partitions):

  PSUM tile = (-2 x_chunk^T)^T @ y^T            (K=128 fp16 matmul)
            + [ones; x2_chunk]^T @ [y2; ones]   (K=2 rank-2 bias fold)
  => PSUM[i, j] = shifted dist (x2/y2 are shifted by their minima; the
     shift S is added back on the host, which keeps the fp16 bias rows
     small and precise).

ScalarE drains PSUM -> SBUF fp16 "E" tiles (one pass, 1 elem/cycle).
VectorE does both reductions in fp16 2x mode:
  - per-x row minima: two chained tensor_tensor_reduce ops per chunk
    (out = min(lo, hi) pair-min, accum_out = running min; the second op's
    reduce is seeded with the first op's accum) -- the whole 8192-wide row
    min costs one read of E at 2 elem/cycle/port and lands straight in the
    parts tile, no halving tree.
  - per-y col minima: elementwise-min accumulation split into TWO
    accumulators, colA (chunks 0-3) and colB (chunks 4-7); chunk 0/4
    drains land directly in colA/colB (no init pass). colA is final after
    chunk 3, so its DMA transposes fire at the top of chunk 4 and hide
    under chunks 4-7; colB's last update (chunk 7) is sliced per 2048
    columns with each slice's transpose fired immediately. The
    cross-partition reduce is then min(colAT, colBT) followed by an
    in-place halving tree, with the last level writing parts directly.
All per-iteration tiles come from pools sized so that in the repeated
timing harness iteration n+1's chunk 0 overlaps iteration n's tail.
Per-core partials ([128, 64] per-y minima + [128, 8] per-chunk row
minima, shifted space) go to DRAM; the tiny global combine (min over
cores, add shift, clamp, sum) runs on the host.
"""

import os
import sys

import numpy as np

sys.path.insert(0, "/opt/trn_rl_repo")
os.environ.setdefault("MYCRO_LOCAL_CACHE", "1")

import concourse.bass as bass
import concourse.bacc as bacc
import concourse.mybir as mybir
import concourse.tile as tile
from concourse.bass_utils import run_bass_kernel_spmd

FP16 = mybir.dt.float16
FP32 = mybir.dt.float32
AF = mybir.ActivationFunctionType
ALU = mybir.AluOpType

NPTS = 8192          # points in x and in y
DIM = 128            # feature dim = matmul contraction K
NCORES = 8
LOCAL = NPTS // NCORES   # 1024 x-rows per core
P = 128              # partitions
CHUNKS = LOCAL // P      # 8 chunks of 128 x-rows per core
JT = 512             # j-tile width (one PSUM bank of fp32)
GROUP = 4            # j-tiles per PSUM pool buffer / ACT drain
NGRP = NPTS // (JT * GROUP)  # 4 drain groups per chunk
NBLK = NPTS // P     # 64 column blocks of 128 y-points
TSL = 2048           # transpose pipeline slice (columns)
NTSL = NPTS // TSL
BPS = TSL // P       # 16 blocks per transpose slice

EBUFS = int(os.environ.get("K_EBUFS", "3"))
ROWTTR = int(os.environ.get("K_ROWTTR", "0"))
TQ2 = int(os.environ.get("K_TQ2", "0"))      # alternate SP/Act DMA queues
KTSL = int(os.environ.get("K_TSL", str(TSL)))  # last-chunk transpose slice
MERGE7 = int(os.environ.get("K_MERGE7", "0"))  # merge A,B in chunk 7, halve
                                               # the transposed volume
TQG = int(os.environ.get("K_TQG", "0"))  # guard Act-queue transposes with a
                                         # 1-elem ACT read of the source slice
NOFOLD = int(os.environ.get("K_NOFOLD", "0"))  # TIMING PROBE ONLY: skip the
                                               # bias-fold matmuls (wrong
                                               # results; halves PE work)
RBIG = 60000.0       # min-reduce init (> any shifted fp16 distance)


def _split_stage(stage: str):
    """'host17' -> ('host', 17); 'pe' -> ('pe', default)."""
    base = stage.rstrip("0123456789")
    nrep = int(stage[len(base):]) if len(base) < len(stage) else None
    return base, nrep


def _build_module(stage: str = "full"):
    """stage base: 'host' (per-core partials, host combine) | 'main'
    (chunk loop only, scalar dummy out) | 'pe' (matmuls only) | 'pedrain'
    (matmuls + ACT drain). A trailing integer repeats the body in-NEFF
    for slope timing (e.g. 'host17')."""
    base, nrep = _split_stage(stage)
    nrep = nrep or 1
    nc = bacc.Bacc(
        "TRN2",
        target_bir_lowering=False,
        debug=False,
        num_devices=NCORES,
    )

    xT2 = nc.dram_tensor("xT2", [P, LOCAL], FP16, kind="ExternalInput")
    yT = nc.dram_tensor("yT", [P, NPTS], FP16, kind="ExternalInput")
    fold_lhsT = nc.dram_tensor("fold_lhsT", [2, LOCAL], FP16, kind="ExternalInput")
    fold_rhs = nc.dram_tensor("fold_rhs", [2, NPTS], FP16, kind="ExternalInput")
    if base == "host":
        # per-core partials: cols 0..63 = per-y minima (shifted space, no
        # clamp -- the global min across cores happens on the host), cols
        # 64..71 = per-chunk row minima (shifted space; host adds shift,
        # clamps per row, sums)
        out = nc.dram_tensor("parts", [P, NBLK + CHUNKS], FP32,
                             kind="ExternalOutput")
    else:
        out = nc.dram_tensor("loss", [1, 1], FP32, kind="ExternalOutput")
    if base == "dbg":
        dbg_colA = nc.dram_tensor("dbg_colA", [P, NPTS], FP16,
                                  kind="ExternalOutput")
        dbg_colB = nc.dram_tensor("dbg_colB", [P, NPTS], FP16,
                                  kind="ExternalOutput")
        dbg_colAT = nc.dram_tensor("dbg_colAT", [P, NBLK, P], FP16,
                                   kind="ExternalOutput")
        dbg_colBT = nc.dram_tensor("dbg_colBT", [P, NBLK, P], FP16,
                                   kind="ExternalOutput")
        dbg_parts = nc.dram_tensor("dbg_parts", [P, NBLK + CHUNKS], FP32,
                                   kind="ExternalOutput")

    with tile.TileContext(nc) as tc:
        with (
            tc.tile_pool(name="const", bufs=1) as cpool,
            tc.tile_pool(name="acc", bufs=1) as apool,
            tc.tile_pool(name="epool", bufs=EBUFS) as epool,
            tc.tile_pool(name="scratch", bufs=1) as spool,
            tc.tile_pool(name="smalls", bufs=2) as smpool,
        ):
            sb_xT2 = cpool.tile([P, LOCAL], FP16, tag="xT2")
            sb_yT = cpool.tile([P, NPTS], FP16, tag="yT")
            sb_flhs = cpool.tile([2, LOCAL], FP16, tag="flhs")
            sb_frhs = cpool.tile([2, NPTS], FP16, tag="frhs")

            nc.sync.dma_start(sb_yT[:], yT[:])
            nc.sync.dma_start(sb_xT2[:], xT2[:])
            nc.sync.dma_start(sb_flhs[:], fold_lhsT[:])
            nc.sync.dma_start(sb_frhs[:], fold_rhs[:])

            # DVE-only scratch: safe to share across reps (WARs are
            # DVE-serial and cost nothing)
            rtree = spool.tile([P, NPTS // 2], FP16, tag="rtree")
            rt256 = spool.tile([P, CHUNKS, 256], FP16, tag="rt256")
            sdump = spool.tile([1, 1], FP16, tag="sdump")
            tq_state = {}

            def _tq_transpose(s, tdst, tsrc, bps):
                """Fire slice s's transpose, alternating HWDGE queues when
                TQ2 is on. TQG adds a 1-elem ACT read of the source slice
                before an Act-queue transpose, forcing the ACT sequencer to
                wait for the slice's DVE write before the doorbell."""
                use_act = TQ2 and s % 2
                if use_act and TQG:
                    nc.scalar.activation(
                        sdump[:], tsrc[0:1, s * KTSL : s * KTSL + 1],
                        AF.Copy,
                    )
                eng = nc.scalar if use_act else nc.sync
                eng.dma_start_transpose(
                    tdst[:, s * bps : (s + 1) * bps, :],
                    tsrc[:, bass.ts(s, KTSL)],
                )
            rtrash = rtree  # TTR pair-min trash: overlay on rtree (DVE-only)
            rowtmp = spool.tile([P, 2], FP16, tag="rowtmp")
            colA = apool.tile([P, NPTS], FP16, tag="colA")
            colB = apool.tile([P, NPTS], FP16, tag="colB")

            def _col_tail(parts_prev, colAT, colBT):
                """Rep-n column tail: merge the transposed accumulators
                in-place into colAT, in-place halving tree over the
                within-block axis (last level writes parts), then the
                parts DMA. Emitted software-pipelined -- after rep n+1's
                chunk-TAILPOS ops -- so the chunk-7 transposes complete
                under useful DVE work instead of a stall. colAT/colBT are
                double-buffered per rep, so this never aliases rep n+1's
                transposes."""
                if not MERGE7:
                    nc.vector.tensor_tensor(
                        colAT[:], colAT[:], colBT[:], op=ALU.min,
                    )
                w = P // 2
                while w >= 2:
                    nc.vector.tensor_tensor(
                        colAT[:, :, 0:w], colAT[:, :, 0:w],
                        colAT[:, :, w : 2 * w], op=ALU.min,
                    )
                    w //= 2
                nc.vector.tensor_tensor(
                    parts_prev[:, 0:NBLK], colAT[:, :, 0],
                    colAT[:, :, 1], op=ALU.min,
                )
                if base != "main":
                    nc.sync.dma_start(out[:], parts_prev[:])

            TAILPOS = int(os.environ.get("K_TAILPOS", "3"))
            pending_tail = None

            with (
                tc.tile_pool(name="tp", bufs=2) as tpool,
                tc.tile_pool(name="psum", bufs=2, space="PSUM") as psum_pool,
            ):
              for rep in range(nrep):
                parts_sb = smpool.tile([P, NBLK + CHUNKS], FP32, tag="psb")
                # transposed accumulators: written by DMA at chunks 3/7 of
                # rep n, read by DVE at rep n's deferred tail (which runs
                # inside rep n+1's chunk loop) -- double-buffered so rep
                # n+1's transposes never collide with rep n's tail
                colAT = tpool.tile([P, NBLK, P], FP16, tag="colAT")
                colBT = colAT if MERGE7 else tpool.tile([P, NBLK, P], FP16,
                                                        tag="colBT")

                def rowslot(c0, c1, p0=0, p1=P):
                    return parts_sb[p0:p1, NBLK + c0 : NBLK + c1]

                def _row_reduce(e_c, c, lo, parts_sb=parts_sb,
                                rowtmp=rowtmp):
                    """Row-min of e_c columns [lo, lo+4096) via one fused
                    tensor_tensor_reduce: out (trash) = pair min, accum =
                    min over the pair-min row. The two halves' accums are
                    combined into parts by a tiny tensor_tensor."""
                    q = NPTS // 4
                    h = lo // (NPTS // 2)
                    nc.vector.tensor_tensor_reduce(
                        rtrash[:, 0:q],
                        e_c[:, lo : lo + q],
                        e_c[:, lo + q : lo + 2 * q],
                        scale=1.0,
                        scalar=RBIG,
                        op0=ALU.min,
                        op1=ALU.min,
                        accum_out=rowtmp[:, h : h + 1],
                    )
                    if h == 1:
                        nc.vector.tensor_tensor(
                            rowslot(c, c + 1), rowtmp[:, 0:1],
                            rowtmp[:, 1:2], op=ALU.min,
                        )

                def _row_tree(e_c, c, lo, rtree=rtree, rt256=rt256):
                    """Fallback row path (no TTR): halving tree into the
                    per-chunk rt256 slot."""
                    q = NPTS // 4
                    half = NPTS // 2
                    nc.vector.tensor_tensor(
                        rtree[:, lo // 2 : lo // 2 + q],
                        e_c[:, lo : lo + q],
                        e_c[:, lo + q : lo + 2 * q], op=ALU.min,
                    )
                    if lo == 0:
                        return
                    hw_ = half
                    while hw_ > JT:
                        h2 = hw_ // 2
                        nc.vector.tensor_tensor(
                            rtree[:, 0:h2], rtree[:, 0:h2],
                            rtree[:, h2:hw_], op=ALU.min,
                        )
                        hw_ = h2
                    nc.vector.tensor_tensor(
                        rt256[:, c, :], rtree[:, 0:256], rtree[:, 256:JT],
                        op=ALU.min,
                    )

                for c in range(CHUNKS):
                    direct = c in (0, 4)
                    acc = colA if c < 4 else colB
                    accT = colAT if c < 4 else colBT
                    if direct and base not in ("pe", "pedrain"):
                        # chunks 0/4: ACT drains straight into the
                        # accumulator -- no separate DVE init pass
                        e_c = acc
                    else:
                        e_c = epool.tile([P, NPTS], FP16, tag="E")
                    csl = bass.ts(c, P)
                    for g in range(NGRP):
                        pt = psum_pool.tile([P, GROUP * JT], FP32, tag="D")
                        # main matmuls of the group share one lhsT load;
                        # the K=2 bias folds share another.
                        for t in range(GROUP):
                            j0 = (g * GROUP + t) * JT
                            nc.tensor.matmul(
                                pt[:, bass.ts(t, JT)],
                                lhsT=sb_xT2[:, csl],
                                rhs=sb_yT[:, j0 : j0 + JT],
                                start=True,
                                stop=NOFOLD == 1,
                            )
                        if not NOFOLD:
                            for t in range(GROUP):
                                j0 = (g * GROUP + t) * JT
                                nc.tensor.matmul(
                                    pt[:, bass.ts(t, JT)],
                                    lhsT=sb_flhs[:, csl],
                                    rhs=sb_frhs[:, j0 : j0 + JT],
                                    start=False,
                                    stop=True,
                                )
                        gsl = bass.ts(g, GROUP * JT)
                        if base == "pe":
                            # keep a consumer so matmuls aren't dead: tiny
                            # copy of one column per group
                            nc.scalar.activation(
                                e_c[:, g : g + 1], pt[:, 0:1], AF.Copy
                            )
                        else:
                            nc.scalar.activation(e_c[:, gsl], pt[:], AF.Copy)

                    if base in ("pe", "pedrain"):
                        # tiny reader keeps each chunk's work live
                        nc.vector.tensor_copy(
                            rowslot(c, c + 1, 0, 1), e_c[0:1, 0:1]
                        )
                        continue

                    last = c == CHUNKS - 1
                    half = NPTS // 2
                    # each accumulator's FINAL update (chunk 3 for colA,
                    # chunk 7 for colB) is sliced per K_TSL cols with the
                    # slice's transpose fired immediately, so the DMA gets
                    # the longest possible window to hide
                    fire = c == 7 if MERGE7 else c in (3, 7)

                    def _acc_half(h, acc=acc, accT=accT, e_c=e_c,
                                  direct=direct, fire=fire):
                        """Accumulator update for column half h (0/1)."""
                        if direct:
                            return
                        if not fire:
                            sl = slice(h * half, (h + 1) * half)
                            nc.vector.tensor_tensor(
                                acc[:, sl], e_c[:, sl], acc[:, sl],
                                op=ALU.min,
                            )
                            return
                        ns2 = half // KTSL  # slices per half
                        bps = KTSL // P
                        for s in range(h * ns2, (h + 1) * ns2):
                            ssl = bass.ts(s, KTSL)
                            nc.vector.tensor_tensor(
                                acc[:, ssl], e_c[:, ssl], acc[:, ssl],
                                op=ALU.min,
                            )
                            if MERGE7:
                                nc.vector.tensor_tensor(
                                    colA[:, ssl], colA[:, ssl],
                                    acc[:, ssl], op=ALU.min,
                                )
                            _tq_transpose(s, colAT if MERGE7 else accT,
                                          colA if MERGE7 else acc, bps)

                    # interleave: acc-lo + row-lo need only drain groups
                    # 0-1; acc-hi + row-hi need groups 2-3
                    _acc_half(0)
                    if ROWTTR:
                        _row_reduce(e_c, c, 0)
                        _acc_half(1)
                        _row_reduce(e_c, c, half)
                    else:
                        _row_tree(e_c, c, 0)
                        _acc_half(1)
                        _row_tree(e_c, c, half)

                    if c == TAILPOS and pending_tail is not None:
                        _col_tail(*pending_tail)
                        pending_tail = None

                if base in ("pe", "pedrain"):
                    continue
                if not ROWTTR:
                    # collapse the per-chunk 256-wide partials: in-place
                    # halving tree on the last axis (2x mode), final level
                    # lands in the parts row slots.
                    w = 128
                    while w >= 2:
                        nc.vector.tensor_tensor(
                            rt256[:, :, 0:w], rt256[:, :, 0:w],
                            rt256[:, :, w : 2 * w], op=ALU.min,
                        )
                        w //= 2
                    nc.vector.tensor_tensor(
                        rowslot(0, CHUNKS), rt256[:, :, 0],
                        rt256[:, :, 1], op=ALU.min,
                    )
                if base == "main":
                    lres0 = spool.tile([1, 1], FP32, tag="lres0")
                    nc.vector.tensor_copy(lres0[:], rowslot(0, 1, 0, 1))
                    nc.sync.dma_start(out[:], lres0[:])
                if base == "dbg":
                    _col_tail(parts_sb, colAT, colBT)
                    nc.sync.dma_start(dbg_parts[:], parts_sb[:])
                    nc.sync.dma_start(dbg_colA[:], colA[:])
                    nc.sync.dma_start(dbg_colB[:], colB[:])
                    nc.sync.dma_start(dbg_colAT[:], colAT[:])
                    nc.sync.dma_start(dbg_colBT[:], colBT[:])
                    lres0 = spool.tile([1, 1], FP32, tag="lres0")
                    nc.vector.tensor_copy(lres0[:], parts_sb[0:1, 0:1])
                    nc.sync.dma_start(out[:], lres0[:])
                else:
                    # column tail + parts DMA are deferred into the next
                    # rep's chunk loop (software pipelining)
                    pending_tail = (parts_sb, colAT, colBT)

              if pending_tail is not None and base not in ("pe", "pedrain"):
                _col_tail(*pending_tail)
                pending_tail = None

            if base in ("pe", "pedrain"):
                lres0 = spool.tile([1, 1], FP32, tag="lres0")
                nc.vector.tensor_copy(lres0[:], parts_sb[0:1, NBLK:NBLK + 1])
                nc.sync.dma_start(out[:], lres0[:])

    nc.compile()
    return nc


_NC_CACHE: dict = {}


def _get_module(stage: str = "host"):
    if stage not in _NC_CACHE:
        _NC_CACHE[stage] = _build_module(stage)
    return _NC_CACHE[stage]


_RUNNER_CACHE: dict = {}


def _get_runner(stage: str = "host", donate: bool = True):
    """Build (once) a jitted SPMD callable over the 8 cores.

    Mirrors concourse.bass2jax.run_bass_via_pjrt but caches the jitted
    function so repeated calls don't re-trace, and exposes the pieces
    needed for device-resident benchmarking.
    """
    key = (stage, donate)
    if key in _RUNNER_CACHE:
        return _RUNNER_CACHE[key]

    import jax
    from jax.sharding import Mesh, PartitionSpec
    from jax.experimental.shard_map import shard_map
    import concourse.mybir as _mybir
    from concourse import bass2jax

    nc = _get_module(stage)
    bass2jax.install_neuronx_cc_hook()

    partition_name = (
        nc.partition_id_tensor.name if nc.partition_id_tensor else None
    )
    in_names: list[str] = []
    out_names: list[str] = []
    out_avals: list[jax.core.ShapedArray] = []
    zero_outs: list[np.ndarray] = []
    for alloc in nc.m.functions[0].allocations:
        if not isinstance(alloc, _mybir.MemoryLocationSet):
            continue
        name = alloc.memorylocations[0].name
        if alloc.kind == "ExternalInput":
            if name != partition_name:
                in_names.append(name)
        elif alloc.kind == "ExternalOutput":
            out_names.append(name)
            shape = tuple(alloc.tensor_shape)
            dtype = _mybir.dt.np(alloc.dtype)
            out_avals.append(jax.core.ShapedArray(shape, dtype))
            zero_outs.append(np.zeros(shape, dtype))
    n_params = len(in_names)
    n_outs = len(out_avals)
    all_names = in_names + out_names
    if partition_name is not None:
        all_names = all_names + [partition_name]

    def _body(*args):
        operands = list(args)
        if partition_name is not None:
            operands.append(bass2jax.partition_id_tensor())
        outs = bass2jax._bass_exec_p.bind(
            *operands,
            out_avals=tuple(out_avals),
            in_names=tuple(all_names),
            out_names=tuple(out_names),
            lowering_input_output_aliases=(),
            sim_require_finite=True,
            sim_require_nnan=True,
            nc=nc,
        )
        return tuple(outs)

    devices = jax.devices()[:NCORES]
    mesh = Mesh(np.asarray(devices), ("core",))
    in_specs = (PartitionSpec("core"),) * (n_params + n_outs)
    out_specs = (PartitionSpec("core"),) * n_outs
    jit_kw = (
        dict(donate_argnums=tuple(range(n_params, n_params + n_outs)))
        if donate
        else {}
    )
    sharded = jax.jit(
        shard_map(_body, mesh=mesh, in_specs=in_specs, out_specs=out_specs,
                  check_rep=False),
        keep_unused=True,
        **jit_kw,
    )
    _RUNNER_CACHE[key] = (sharded, in_names, out_names, out_avals, zero_outs,
                          mesh)
    return _RUNNER_CACHE[key]


def _run(in_maps, stage="host"):
    sharded, in_names, out_names, out_avals, zero_outs, _ = _get_runner(stage)
    concat_in = [
        np.concatenate([np.asarray(in_maps[c][n]) for c in range(NCORES)], axis=0)
        for n in in_names
    ]
    concat_zeros = [
        np.zeros((NCORES * z.shape[0], *z.shape[1:]), z.dtype) for z in zero_outs
    ]
    out_arrs = sharded(*concat_in, *concat_zeros)
    return [
        {
            n: np.asarray(out_arrs[i]).reshape(NCORES, *out_avals[i].shape)[c]
            for i, n in enumerate(out_names)
        }
        for c in range(NCORES)
    ]


def _prep_inputs(x: np.ndarray, y: np.ndarray):
    return _prep_inputs_s(x, y)[0]


def _prep_inputs_s(x: np.ndarray, y: np.ndarray):
    x = np.asarray(x, np.float32)
    y = np.asarray(y, np.float32)
    x2 = np.sum(x.astype(np.float64) ** 2, axis=1)
    y2 = np.sum(y.astype(np.float64) ** 2, axis=1)
    s = float(x2.min() + y2.min())
    x2s = (x2 - x2.min()).astype(np.float32)
    y2s = (y2 - y2.min()).astype(np.float32)

    yT = np.ascontiguousarray(y.T).astype(np.float16)
    fold_rhs = np.empty((2, NPTS), np.float16)
    fold_rhs[0] = y2s.astype(np.float16)
    fold_rhs[1] = 1.0

    in_maps = []
    for c in range(NCORES):
        sl = slice(c * LOCAL, (c + 1) * LOCAL)
        xT2 = np.ascontiguousarray((-2.0 * x[sl]).T).astype(np.float16)
        fold_lhsT = np.empty((2, LOCAL), np.float16)
        fold_lhsT[0] = 1.0
        fold_lhsT[1] = x2s[sl].astype(np.float16)
        in_maps.append(
            {
                "xT2": xT2,
                "yT": yT,
                "fold_lhsT": fold_lhsT,
                "fold_rhs": fold_rhs,
            }
        )
    return in_maps, s


def kernel(x: np.ndarray, y: np.ndarray, **_ignored):
    x = np.asarray(x, np.float32)
    y = np.asarray(y, np.float32)
    in_maps, s = _prep_inputs_s(x, y)
    results = _run(in_maps, stage="host")
    parts = np.stack([results[c]["parts"] for c in range(NCORES)])  # [8,128,72]
    colmin = parts[:, :, 0:NBLK].min(axis=0)       # global per-y minima
    s2 = np.maximum(colmin.astype(np.float64) + s, 0.0).sum()
    rows = parts[:, :, NBLK:].astype(np.float64)   # per-chunk row minima
    s1 = np.maximum(rows + s, 0.0).sum()
    return np.float32((s1 + s2) / NPTS)


# revision 44
# speedup vs baseline: 1.1543x; 1.1543x over previous
"""Chamfer loss kernel for Trainium2 (8 NeuronCores, Bass/Tile).

Problem: x, y of shape [8192, 128] fp32.
  dist[i,j] = max(||x_i||^2 + ||y_j||^2 - 2 x_i.y_j, 0)
  loss = (sum_j min_i dist[i,j] + sum_i min_j dist[i,j]) / 8192

Sharding: x rows are split across the 8 cores (1024 rows each); every core
holds all of y. Each core computes its [1024, 8192] distance tile via PE
matmuls (K = 128 features on partitions):

  PSUM tile = (-2 x_chunk^T)^T @ y^T            (K=128 fp16 matmul)
            + [ones; x2_chunk]^T @ [y2; ones]   (K=2 rank-2 bias fold)
  => PSUM[i, j] = shifted dist (x2/y2 are shifted by their minima; the
     shift S is added back on the host, which keeps the fp16 bias rows
     small and precise).

ScalarE drains PSUM -> SBUF fp16 "E" tiles (one pass, 1 elem/cycle).
VectorE does both reductions in fp16 2x mode:
  - per-x row minima: two chained tensor_tensor_reduce ops per chunk
    (out = min(lo, hi) pair-min, accum_out = running min; the second op's
    reduce is seeded with the first op's accum) -- the whole 8192-wide row
    min costs one read of E at 2 elem/cycle/port and lands straight in the
    parts tile, no halving tree.
  - per-y col minima: elementwise-min accumulation split into TWO
    accumulators, colA (chunks 0-3) and colB (chunks 4-7); chunk 0/4
    drains land directly in colA/colB (no init pass). colA is final after
    chunk 3, so its DMA transposes fire at the top of chunk 4 and hide
    under chunks 4-7; colB's last update (chunk 7) is sliced per 2048
    columns with each slice's transpose fired immediately. The
    cross-partition reduce is then min(colAT, colBT) followed by an
    in-place halving tree, with the last level writing parts directly.
All per-iteration tiles come from pools sized so that in the repeated
timing harness iteration n+1's chunk 0 overlaps iteration n's tail.
Per-core partials ([128, 64] per-y minima + [128, 8] per-chunk row
minima, shifted space) go to DRAM; the tiny global combine (min over
cores, add shift, clamp, sum) runs on the host.
"""

import os
import sys

import numpy as np

sys.path.insert(0, "/opt/trn_rl_repo")
os.environ.setdefault("MYCRO_LOCAL_CACHE", "1")

import concourse.bass as bass
import concourse.bacc as bacc
import concourse.mybir as mybir
import concourse.tile as tile
from concourse.bass_utils import run_bass_kernel_spmd

FP16 = mybir.dt.float16
FP32 = mybir.dt.float32
AF = mybir.ActivationFunctionType
ALU = mybir.AluOpType

NPTS = 8192          # points in x and in y
DIM = 128            # feature dim = matmul contraction K
NCORES = 8
LOCAL = NPTS // NCORES   # 1024 x-rows per core
P = 128              # partitions
CHUNKS = LOCAL // P      # 8 chunks of 128 x-rows per core
JT = 512             # j-tile width (one PSUM bank of fp32)
GROUP = 4            # j-tiles per PSUM pool buffer / ACT drain
NGRP = NPTS // (JT * GROUP)  # 4 drain groups per chunk
NBLK = NPTS // P     # 64 column blocks of 128 y-points
TSL = 2048           # transpose pipeline slice (columns)
NTSL = NPTS // TSL
BPS = TSL // P       # 16 blocks per transpose slice

EBUFS = int(os.environ.get("K_EBUFS", "4"))
ROWTTR = int(os.environ.get("K_ROWTTR", "0"))
TQ2 = int(os.environ.get("K_TQ2", "0"))      # alternate SP/Act DMA queues
KTSL = int(os.environ.get("K_TSL", str(TSL)))  # last-chunk transpose slice
MERGE7 = int(os.environ.get("K_MERGE7", "0"))  # merge A,B in chunk 7, halve
                                               # the transposed volume
TQG = int(os.environ.get("K_TQG", "0"))  # guard Act-queue transposes with a
                                         # 1-elem ACT read of the source slice
NOFOLD = int(os.environ.get("K_NOFOLD", "0"))  # TIMING PROBE ONLY: skip the
                                               # bias-fold matmuls (wrong
                                               # results; halves PE work)
RBIG = 60000.0       # min-reduce init (> any shifted fp16 distance)


def _split_stage(stage: str):
    """'host17' -> ('host', 17); 'pe' -> ('pe', default)."""
    base = stage.rstrip("0123456789")
    nrep = int(stage[len(base):]) if len(base) < len(stage) else None
    return base, nrep


def _build_module(stage: str = "full"):
    """stage base: 'host' (per-core partials, host combine) | 'main'
    (chunk loop only, scalar dummy out) | 'pe' (matmuls only) | 'pedrain'
    (matmuls + ACT drain). A trailing integer repeats the body in-NEFF
    for slope timing (e.g. 'host17')."""
    base, nrep = _split_stage(stage)
    nrep = nrep or 1
    nc = bacc.Bacc(
        "TRN2",
        target_bir_lowering=False,
        debug=False,
        num_devices=NCORES,
    )

    xT2 = nc.dram_tensor("xT2", [P, LOCAL], FP16, kind="ExternalInput")
    yT = nc.dram_tensor("yT", [P, NPTS], FP16, kind="ExternalInput")
    fold_lhsT = nc.dram_tensor("fold_lhsT", [2, LOCAL], FP16, kind="ExternalInput")
    fold_rhs = nc.dram_tensor("fold_rhs", [2, NPTS], FP16, kind="ExternalInput")
    if base == "host":
        # per-core partials: cols 0..63 = per-y minima (shifted space, no
        # clamp -- the global min across cores happens on the host), cols
        # 64..71 = per-chunk row minima (shifted space; host adds shift,
        # clamps per row, sums)
        out = nc.dram_tensor("parts", [P, NBLK + CHUNKS], FP32,
                             kind="ExternalOutput")
    else:
        out = nc.dram_tensor("loss", [1, 1], FP32, kind="ExternalOutput")
    if base == "dbg":
        dbg_colA = nc.dram_tensor("dbg_colA", [P, NPTS], FP16,
                                  kind="ExternalOutput")
        dbg_colB = nc.dram_tensor("dbg_colB", [P, NPTS], FP16,
                                  kind="ExternalOutput")
        dbg_colAT = nc.dram_tensor("dbg_colAT", [P, NBLK, P], FP16,
                                   kind="ExternalOutput")
        dbg_colBT = nc.dram_tensor("dbg_colBT", [P, NBLK, P], FP16,
                                   kind="ExternalOutput")
        dbg_parts = nc.dram_tensor("dbg_parts", [P, NBLK + CHUNKS], FP32,
                                   kind="ExternalOutput")

    with tile.TileContext(nc) as tc:
        with (
            tc.tile_pool(name="const", bufs=1) as cpool,
            tc.tile_pool(name="acc", bufs=1) as apool,
            tc.tile_pool(name="epool", bufs=EBUFS) as epool,
            tc.tile_pool(name="scratch", bufs=1) as spool,
            tc.tile_pool(name="smalls", bufs=2) as smpool,
        ):
            sb_xT2 = cpool.tile([P, LOCAL], FP16, tag="xT2")
            sb_yT = cpool.tile([P, NPTS], FP16, tag="yT")
            sb_flhs = cpool.tile([2, LOCAL], FP16, tag="flhs")
            sb_frhs = cpool.tile([2, NPTS], FP16, tag="frhs")

            nc.sync.dma_start(sb_yT[:], yT[:])
            nc.sync.dma_start(sb_xT2[:], xT2[:])
            nc.sync.dma_start(sb_flhs[:], fold_lhsT[:])
            nc.sync.dma_start(sb_frhs[:], fold_rhs[:])

            # DVE-only scratch: safe to share across reps (WARs are
            # DVE-serial and cost nothing)
            rtree = spool.tile([P, NPTS // 2], FP16, tag="rtree")
            rt256 = spool.tile([P, CHUNKS, 256], FP16, tag="rt256")
            sdump = spool.tile([1, 1], FP16, tag="sdump")
            tq_state = {}

            def _tq_transpose(s, tdst, tsrc, bps):
                """Fire slice s's transpose, alternating HWDGE queues when
                TQ2 is on. TQG adds a 1-elem ACT read of the source slice
                before an Act-queue transpose, forcing the ACT sequencer to
                wait for the slice's DVE write before the doorbell."""
                use_act = TQ2 and s % 2
                if use_act and TQG:
                    nc.scalar.activation(
                        sdump[:], tsrc[0:1, s * KTSL : s * KTSL + 1],
                        AF.Copy,
                    )
                eng = nc.scalar if use_act else nc.sync
                eng.dma_start_transpose(
                    tdst[:, s * bps : (s + 1) * bps, :],
                    tsrc[:, bass.ts(s, KTSL)],
                )
            rtrash = rtree  # TTR pair-min trash: overlay on rtree (DVE-only)
            rowtmp = spool.tile([P, 2], FP16, tag="rowtmp")
            colA = apool.tile([P, NPTS], FP16, tag="colA")
            colB = apool.tile([P, NPTS], FP16, tag="colB")

            def _col_tail(parts_prev, colAT, colBT):
                """Rep-n column tail: merge the transposed accumulators
                in-place into colAT, in-place halving tree over the
                within-block axis (last level writes parts), then the
                parts DMA. Emitted software-pipelined -- after rep n+1's
                chunk-TAILPOS ops -- so the chunk-7 transposes complete
                under useful DVE work instead of a stall. colAT/colBT are
                double-buffered per rep, so this never aliases rep n+1's
                transposes."""
                if not MERGE7:
                    nc.vector.tensor_tensor(
                        colAT[:], colAT[:], colBT[:], op=ALU.min,
                    )
                w = P // 2
                while w >= 2:
                    nc.vector.tensor_tensor(
                        colAT[:, :, 0:w], colAT[:, :, 0:w],
                        colAT[:, :, w : 2 * w], op=ALU.min,
                    )
                    w //= 2
                nc.vector.tensor_tensor(
                    parts_prev[:, 0:NBLK], colAT[:, :, 0],
                    colAT[:, :, 1], op=ALU.min,
                )
                if base != "main":
                    nc.sync.dma_start(out[:], parts_prev[:])

            TAILPOS = int(os.environ.get("K_TAILPOS", "2"))
            TBUFS = int(os.environ.get("K_TBUFS", "1"))
            pending_tail = None

            with (
                tc.tile_pool(name="tp", bufs=TBUFS) as tpool,
                tc.tile_pool(name="psum", bufs=2, space="PSUM") as psum_pool,
            ):
              for rep in range(nrep):
                parts_sb = smpool.tile([P, NBLK + CHUNKS], FP32, tag="psb")
                # transposed accumulators: written by DMA at chunks 3/7 of
                # rep n, read by DVE at rep n's deferred tail (which runs
                # inside rep n+1's chunk loop) -- double-buffered so rep
                # n+1's transposes never collide with rep n's tail
                colAT = tpool.tile([P, NBLK, P], FP16, tag="colAT")
                colBT = colAT if MERGE7 else tpool.tile([P, NBLK, P], FP16,
                                                        tag="colBT")

                def rowslot(c0, c1, p0=0, p1=P):
                    return parts_sb[p0:p1, NBLK + c0 : NBLK + c1]

                def _row_reduce(e_c, c, lo, parts_sb=parts_sb,
                                rowtmp=rowtmp):
                    """Row-min of e_c columns [lo, lo+4096) via one fused
                    tensor_tensor_reduce: out (trash) = pair min, accum =
                    min over the pair-min row. The two halves' accums are
                    combined into parts by a tiny tensor_tensor."""
                    q = NPTS // 4
                    h = lo // (NPTS // 2)
                    nc.vector.tensor_tensor_reduce(
                        rtrash[:, 0:q],
                        e_c[:, lo : lo + q],
                        e_c[:, lo + q : lo + 2 * q],
                        scale=1.0,
                        scalar=RBIG,
                        op0=ALU.min,
                        op1=ALU.min,
                        accum_out=rowtmp[:, h : h + 1],
                    )
                    if h == 1:
                        nc.vector.tensor_tensor(
                            rowslot(c, c + 1), rowtmp[:, 0:1],
                            rowtmp[:, 1:2], op=ALU.min,
                        )

                def _row_tree(e_c, c, lo, rtree=rtree, rt256=rt256):
                    """Fallback row path (no TTR): halving tree into the
                    per-chunk rt256 slot."""
                    q = NPTS // 4
                    half = NPTS // 2
                    nc.vector.tensor_tensor(
                        rtree[:, lo // 2 : lo // 2 + q],
                        e_c[:, lo : lo + q],
                        e_c[:, lo + q : lo + 2 * q], op=ALU.min,
                    )
                    if lo == 0:
                        return
                    hw_ = half
                    while hw_ > JT:
                        h2 = hw_ // 2
                        nc.vector.tensor_tensor(
                            rtree[:, 0:h2], rtree[:, 0:h2],
                            rtree[:, h2:hw_], op=ALU.min,
                        )
                        hw_ = h2
                    nc.vector.tensor_tensor(
                        rt256[:, c, :], rtree[:, 0:256], rtree[:, 256:JT],
                        op=ALU.min,
                    )

                for c in range(CHUNKS):
                    direct = c in (0, 4)
                    acc = colA if c < 4 else colB
                    accT = colAT if c < 4 else colBT
                    if direct and base not in ("pe", "pedrain"):
                        # chunks 0/4: ACT drains straight into the
                        # accumulator -- no separate DVE init pass
                        e_c = acc
                    else:
                        e_c = epool.tile([P, NPTS], FP16, tag="E")
                    csl = bass.ts(c, P)
                    for g in range(NGRP):
                        pt = psum_pool.tile([P, GROUP * JT], FP32, tag="D")
                        # main matmuls of the group share one lhsT load;
                        # the K=2 bias folds share another.
                        for t in range(GROUP):
                            j0 = (g * GROUP + t) * JT
                            nc.tensor.matmul(
                                pt[:, bass.ts(t, JT)],
                                lhsT=sb_xT2[:, csl],
                                rhs=sb_yT[:, j0 : j0 + JT],
                                start=True,
                                stop=NOFOLD == 1,
                            )
                        if not NOFOLD:
                            for t in range(GROUP):
                                j0 = (g * GROUP + t) * JT
                                nc.tensor.matmul(
                                    pt[:, bass.ts(t, JT)],
                                    lhsT=sb_flhs[:, csl],
                                    rhs=sb_frhs[:, j0 : j0 + JT],
                                    start=False,
                                    stop=True,
                                )
                        gsl = bass.ts(g, GROUP * JT)
                        if base == "pe":
                            # keep a consumer so matmuls aren't dead: tiny
                            # copy of one column per group
                            nc.scalar.activation(
                                e_c[:, g : g + 1], pt[:, 0:1], AF.Copy
                            )
                        else:
                            nc.scalar.activation(e_c[:, gsl], pt[:], AF.Copy)

                    if base in ("pe", "pedrain"):
                        # tiny reader keeps each chunk's work live
                        nc.vector.tensor_copy(
                            rowslot(c, c + 1, 0, 1), e_c[0:1, 0:1]
                        )
                        continue

                    last = c == CHUNKS - 1
                    half = NPTS // 2
                    # each accumulator's FINAL update (chunk 3 for colA,
                    # chunk 7 for colB) is sliced per K_TSL cols with the
                    # slice's transpose fired immediately, so the DMA gets
                    # the longest possible window to hide
                    fire = c == 7 if MERGE7 else c in (3, 7)

                    def _acc_half(h, acc=acc, accT=accT, e_c=e_c,
                                  direct=direct, fire=fire):
                        """Accumulator update for column half h (0/1)."""
                        if direct:
                            return
                        if not fire:
                            sl = slice(h * half, (h + 1) * half)
                            nc.vector.tensor_tensor(
                                acc[:, sl], e_c[:, sl], acc[:, sl],
                                op=ALU.min,
                            )
                            return
                        ns2 = half // KTSL  # slices per half
                        bps = KTSL // P
                        for s in range(h * ns2, (h + 1) * ns2):
                            ssl = bass.ts(s, KTSL)
                            nc.vector.tensor_tensor(
                                acc[:, ssl], e_c[:, ssl], acc[:, ssl],
                                op=ALU.min,
                            )
                            if MERGE7:
                                nc.vector.tensor_tensor(
                                    colA[:, ssl], colA[:, ssl],
                                    acc[:, ssl], op=ALU.min,
                                )
                            _tq_transpose(s, colAT if MERGE7 else accT,
                                          colA if MERGE7 else acc, bps)

                    # interleave: acc-lo + row-lo need only drain groups
                    # 0-1; acc-hi + row-hi need groups 2-3
                    _acc_half(0)
                    if ROWTTR:
                        _row_reduce(e_c, c, 0)
                        _acc_half(1)
                        _row_reduce(e_c, c, half)
                    else:
                        _row_tree(e_c, c, 0)
                        _acc_half(1)
                        _row_tree(e_c, c, half)

                    if c == TAILPOS and pending_tail is not None:
                        _col_tail(*pending_tail)
                        pending_tail = None

                if base in ("pe", "pedrain"):
                    continue
                if not ROWTTR:
                    # collapse the per-chunk 256-wide partials: in-place
                    # halving tree on the last axis (2x mode), final level
                    # lands in the parts row slots.
                    w = 128
                    while w >= 2:
                        nc.vector.tensor_tensor(
                            rt256[:, :, 0:w], rt256[:, :, 0:w],
                            rt256[:, :, w : 2 * w], op=ALU.min,
                        )
                        w //= 2
                    nc.vector.tensor_tensor(
                        rowslot(0, CHUNKS), rt256[:, :, 0],
                        rt256[:, :, 1], op=ALU.min,
                    )
                if base == "main":
                    lres0 = spool.tile([1, 1], FP32, tag="lres0")
                    nc.vector.tensor_copy(lres0[:], rowslot(0, 1, 0, 1))
                    nc.sync.dma_start(out[:], lres0[:])
                if base == "dbg":
                    _col_tail(parts_sb, colAT, colBT)
                    nc.sync.dma_start(dbg_parts[:], parts_sb[:])
                    nc.sync.dma_start(dbg_colA[:], colA[:])
                    nc.sync.dma_start(dbg_colB[:], colB[:])
                    nc.sync.dma_start(dbg_colAT[:], colAT[:])
                    nc.sync.dma_start(dbg_colBT[:], colBT[:])
                    lres0 = spool.tile([1, 1], FP32, tag="lres0")
                    nc.vector.tensor_copy(lres0[:], parts_sb[0:1, 0:1])
                    nc.sync.dma_start(out[:], lres0[:])
                else:
                    # column tail + parts DMA are deferred into the next
                    # rep's chunk loop (software pipelining)
                    pending_tail = (parts_sb, colAT, colBT)

              if pending_tail is not None and base not in ("pe", "pedrain"):
                _col_tail(*pending_tail)
                pending_tail = None

            if base in ("pe", "pedrain"):
                lres0 = spool.tile([1, 1], FP32, tag="lres0")
                nc.vector.tensor_copy(lres0[:], parts_sb[0:1, NBLK:NBLK + 1])
                nc.sync.dma_start(out[:], lres0[:])

    nc.compile()
    return nc


_NC_CACHE: dict = {}


def _get_module(stage: str = "host"):
    if stage not in _NC_CACHE:
        _NC_CACHE[stage] = _build_module(stage)
    return _NC_CACHE[stage]


_RUNNER_CACHE: dict = {}


def _get_runner(stage: str = "host", donate: bool = True):
    """Build (once) a jitted SPMD callable over the 8 cores.

    Mirrors concourse.bass2jax.run_bass_via_pjrt but caches the jitted
    function so repeated calls don't re-trace, and exposes the pieces
    needed for device-resident benchmarking.
    """
    key = (stage, donate)
    if key in _RUNNER_CACHE:
        return _RUNNER_CACHE[key]

    import jax
    from jax.sharding import Mesh, PartitionSpec
    from jax.experimental.shard_map import shard_map
    import concourse.mybir as _mybir
    from concourse import bass2jax

    nc = _get_module(stage)
    bass2jax.install_neuronx_cc_hook()

    partition_name = (
        nc.partition_id_tensor.name if nc.partition_id_tensor else None
    )
    in_names: list[str] = []
    out_names: list[str] = []
    out_avals: list[jax.core.ShapedArray] = []
    zero_outs: list[np.ndarray] = []
    for alloc in nc.m.functions[0].allocations:
        if not isinstance(alloc, _mybir.MemoryLocationSet):
            continue
        name = alloc.memorylocations[0].name
        if alloc.kind == "ExternalInput":
            if name != partition_name:
                in_names.append(name)
        elif alloc.kind == "ExternalOutput":
            out_names.append(name)
            shape = tuple(alloc.tensor_shape)
            dtype = _mybir.dt.np(alloc.dtype)
            out_avals.append(jax.core.ShapedArray(shape, dtype))
            zero_outs.append(np.zeros(shape, dtype))
    n_params = len(in_names)
    n_outs = len(out_avals)
    all_names = in_names + out_names
    if partition_name is not None:
        all_names = all_names + [partition_name]

    def _body(*args):
        operands = list(args)
        if partition_name is not None:
            operands.append(bass2jax.partition_id_tensor())
        outs = bass2jax._bass_exec_p.bind(
            *operands,
            out_avals=tuple(out_avals),
            in_names=tuple(all_names),
            out_names=tuple(out_names),
            lowering_input_output_aliases=(),
            sim_require_finite=True,
            sim_require_nnan=True,
            nc=nc,
        )
        return tuple(outs)

    devices = jax.devices()[:NCORES]
    mesh = Mesh(np.asarray(devices), ("core",))
    in_specs = (PartitionSpec("core"),) * (n_params + n_outs)
    out_specs = (PartitionSpec("core"),) * n_outs
    jit_kw = (
        dict(donate_argnums=tuple(range(n_params, n_params + n_outs)))
        if donate
        else {}
    )
    sharded = jax.jit(
        shard_map(_body, mesh=mesh, in_specs=in_specs, out_specs=out_specs,
                  check_rep=False),
        keep_unused=True,
        **jit_kw,
    )
    _RUNNER_CACHE[key] = (sharded, in_names, out_names, out_avals, zero_outs,
                          mesh)
    return _RUNNER_CACHE[key]


def _run(in_maps, stage="host"):
    sharded, in_names, out_names, out_avals, zero_outs, _ = _get_runner(stage)
    concat_in = [
        np.concatenate([np.asarray(in_maps[c][n]) for c in range(NCORES)], axis=0)
        for n in in_names
    ]
    concat_zeros = [
        np.zeros((NCORES * z.shape[0], *z.shape[1:]), z.dtype) for z in zero_outs
    ]
    out_arrs = sharded(*concat_in, *concat_zeros)
    return [
        {
            n: np.asarray(out_arrs[i]).reshape(NCORES, *out_avals[i].shape)[c]
            for i, n in enumerate(out_names)
        }
        for c in range(NCORES)
    ]


def _prep_inputs(x: np.ndarray, y: np.ndarray):
    return _prep_inputs_s(x, y)[0]


def _prep_inputs_s(x: np.ndarray, y: np.ndarray):
    x = np.asarray(x, np.float32)
    y = np.asarray(y, np.float32)
    x2 = np.sum(x.astype(np.float64) ** 2, axis=1)
    y2 = np.sum(y.astype(np.float64) ** 2, axis=1)
    s = float(x2.min() + y2.min())
    x2s = (x2 - x2.min()).astype(np.float32)
    y2s = (y2 - y2.min()).astype(np.float32)

    yT = np.ascontiguousarray(y.T).astype(np.float16)
    fold_rhs = np.empty((2, NPTS), np.float16)
    fold_rhs[0] = y2s.astype(np.float16)
    fold_rhs[1] = 1.0

    in_maps = []
    for c in range(NCORES):
        sl = slice(c * LOCAL, (c + 1) * LOCAL)
        xT2 = np.ascontiguousarray((-2.0 * x[sl]).T).astype(np.float16)
        fold_lhsT = np.empty((2, LOCAL), np.float16)
        fold_lhsT[0] = 1.0
        fold_lhsT[1] = x2s[sl].astype(np.float16)
        in_maps.append(
            {
                "xT2": xT2,
                "yT": yT,
                "fold_lhsT": fold_lhsT,
                "fold_rhs": fold_rhs,
            }
        )
    return in_maps, s


def kernel(x: np.ndarray, y: np.ndarray, **_ignored):
    x = np.asarray(x, np.float32)
    y = np.asarray(y, np.float32)
    in_maps, s = _prep_inputs_s(x, y)
    results = _run(in_maps, stage="host")
    parts = np.stack([results[c]["parts"] for c in range(NCORES)])  # [8,128,72]
    colmin = parts[:, :, 0:NBLK].min(axis=0)       # global per-y minima
    s2 = np.maximum(colmin.astype(np.float64) + s, 0.0).sum()
    rows = parts[:, :, NBLK:].astype(np.float64)   # per-chunk row minima
    s1 = np.maximum(rows + s, 0.0).sum()
    return np.float32((s1 + s2) / NPTS)
